# revision 1
# baseline (speedup 1.0000x reference)
# DKVMN Trainium2 Bass kernel.
#
# Sharding: data-parallel over batch across 8 NeuronCores (8 sequences each);
# embedding tables and all parameters replicated.
#
# Per-core program (bs = t*8 + b, "t-major", BS=1600):
#   P1  q2c_table/q2c_mask rows gathered by question id (gpsimd ap_gather,
#       tables SBUF-resident, transposed+combined).
#   P2  index math on DVE/ACT: v_idx = cid + 500*correct; masked entries are
#       redirected to an all-zero pad column so the masked mean needs no
#       per-element mask multiply.
#   P3  index lists rewrapped to ap_gather's [16-partition, col] layout via a
#       DRAM bounce (2 DMAs per index tensor).
#   P4  key/value embedding gathers from SBUF-transposed tables ([128d, C]).
#   P5  concept-mean: j-sum by strided DVE reduce, 1/den broadcast by gpsimd
#       partition_broadcast, fused into bf16 kbar/vbar.
#   P6  w = softmax(kbar^T Mk^T) per 128-row chunk (PE matmul + ACT exp with
#       negated-max bias + accum_out, DVE reciprocal+scale).
#   P7  e/a = sigmoid/tanh(vbar^T W^T + b) (PE + ACT).
#   P8  recurrence over 200 steps, state Mv [128(d), 50(n)x8(b)] bf16 in SBUF:
#       gpsimd broadcasts w_t across partitions; DVE does
#       p0=Mv*w, read=reduce_n(p0), t1=-p0*e, t2=w*a, Mv+=t1+t2
#       with e/a consumed through stride-0 broadcast APs (no materialization).
#   P9  f = tanh([reads, kbar] f_W^T + f_b); out = sigmoid(f p_W^T + p_b).
import sys

for _p in ("/opt/trn_rl_repo", "/root/.axon_site/_ro/trn_rl_repo"):
    if _p not in sys.path:
        sys.path.append(_p)

from contextlib import ExitStack

import numpy as np
import ml_dtypes

import concourse.bass as bass
import concourse.bacc as bacc
import concourse.mybir as mybir
from concourse.bass_utils import run_bass_kernel_spmd
from concourse.tile import TileContext

F32 = mybir.dt.float32
BF16 = mybir.dt.bfloat16
I32 = mybir.dt.int32
I16 = mybir.dt.int16
AF = mybir.ActivationFunctionType
OP = mybir.AluOpType

B, S, DK, SLOTS = 64, 200, 128, 50
NUM_Q, NUM_C, MAXC = 10000, 500, 4
NCORES = 8
BL = B // NCORES          # 8 sequences per core
BS = BL * S               # 1600 (bs = t*BL + b)
KPAD = 512                # key table padded cols; zero col at index 500
VPAD = 1008               # value table padded cols; zero col at index 1000
NB = SLOTS * BL           # 400 state columns (n-major, b-inner)
QPAD = ((BS + 127) // 128) * 128   # 1664: q2c gather padded index count

_PROG = None  # cached compiled program


def _build_program():
    nc = bacc.Bacc("TRN2", target_bir_lowering=False, debug=False,
                   num_devices=NCORES)

    def din(name, shape, dt):
        return nc.dram_tensor(name, shape, dt, kind="ExternalInput")

    qseq_w = din("qseq_w", [16, BS // 16], I16)
    corrf = din("corrf", [4, BS], F32)
    q2c_comb = din("q2c_comb", [16, 2 * NUM_Q], I16)
    kt_d = din("kt", [DK, KPAD], F32)
    vt_d = din("vt", [DK, VPAD], F32)
    mkt_d = din("mkt", [DK, SLOTS], BF16)
    ewt_d = din("ewt", [DK, DK], BF16)
    awt_d = din("awt", [DK, DK], BF16)
    fw1t_d = din("fw1t", [DK, DK], BF16)
    fw2t_d = din("fw2t", [DK, DK], BF16)
    pwt_d = din("pwt", [DK, 1], BF16)
    eb_d = din("eb", [DK, 1], F32)
    ab_d = din("ab", [DK, 1], F32)
    fb_d = din("fb", [DK, 1], F32)
    pb_d = din("pb", [1, 1], F32)
    mv0_d = din("mv0r", [DK, NB], BF16)
    out_d = nc.dram_tensor("out", [1, BS], F32, kind="ExternalOutput")

    NCH = (BS + 127) // 128  # 13 bs-chunks (last is 64 rows)

    with ExitStack() as ctx:
        ctx.enter_context(
            nc.allow_low_precision("bf16 state; rel-err budget 2e-2"))
        tc = ctx.enter_context(TileContext(nc))
        const = ctx.enter_context(tc.tile_pool(name="const", bufs=1))
        main = ctx.enter_context(tc.tile_pool(name="main", bufs=1))
        dram = ctx.enter_context(tc.tile_pool(name="dram", bufs=1,
                                              space="DRAM"))

        # ---- persistent tiles ----
        kbar = main.tile([DK, BS], BF16, tag="kbar")
        vbar = main.tile([DK, BS], BF16, tag="vbar")
        e_all = main.tile([DK, BS], BF16, tag="e_all")
        a_all = main.tile([DK, BS], BF16, tag="a_all")
        w_rows = main.tile([128, NCH, SLOTS], BF16, tag="w_rows")
        w32 = main.tile([128, (S + 2) // 3, BL * SLOTS], BF16, tag="w32")
        reads_bf = main.tile([DK, BS], BF16, tag="reads_bf")
        f_all = main.tile([DK, BS], BF16, tag="f_all")
        out_sb = main.tile([1, BS], F32, tag="out_sb")

        # ---- load params (const pool, alive whole kernel) ----
        kt = const.tile([DK, KPAD], F32, tag="kt")
        vt = const.tile([DK, VPAD], F32, tag="vt")
        mkt = const.tile([DK, SLOTS], BF16, tag="mkt")
        ewt = const.tile([DK, DK], BF16, tag="ewt")
        awt = const.tile([DK, DK], BF16, tag="awt")
        fw1t = const.tile([DK, DK], BF16, tag="fw1t")
        fw2t = const.tile([DK, DK], BF16, tag="fw2t")
        pwt = const.tile([DK, 1], BF16, tag="pwt")
        eb = const.tile([DK, 1], F32, tag="eb")
        ab = const.tile([DK, 1], F32, tag="ab")
        fb = const.tile([DK, 1], F32, tag="fb")
        pb = const.tile([1, 1], F32, tag="pb")
        ones4 = const.tile([4, 1], F32, tag="ones4")
        nc.vector.memset(ones4[...], 1.0)
        for tile_, dt_ in ((kt, kt_d), (vt, vt_d), (mkt, mkt_d),
                           (ewt, ewt_d), (awt, awt_d), (fw1t, fw1t_d),
                           (fw2t, fw2t_d), (pwt, pwt_d), (eb, eb_d),
                           (ab, ab_d), (fb, fb_d), (pb, pb_d)):
            nc.sync.dma_start(tile_[...], dt_[...])

        psA_stack = ExitStack()
        psA = psA_stack.enter_context(
            tc.tile_pool(name="psA", bufs=1, space="PSUM"))

        idb = main.tile([DK, BS], BF16, tag="idb")
        kwrap = main.tile([128, BS * 4 // 16], I16, tag="kwrap")
        vwrap = main.tile([128, BS * 4 // 16], I16, tag="vwrap")

        with tc.tile_pool(name="pq", bufs=1) as pq:
            # ---- P1: gather cids/mask rows by question id ----
            q2c_t = pq.tile([16, NUM_Q, 2], I16, tag="q2c")
            qw = pq.tile([16, BS // 16], I16, tag="qw")
            nc.sync.dma_start(q2c_t[...], q2c_comb[...])
            nc.sync.dma_start(qw[...], qseq_w[...])
            qc = pq.tile([16, BS, 2], I16, tag="qc")
            nc.gpsimd.ap_gather(qc[...], q2c_t[...], qw[...], channels=16,
                                num_elems=NUM_Q, d=2, num_idxs=BS)

            # ---- P2: index math (f32, exact for values < 2^24) ----
            corr = pq.tile([4, BS], F32, tag="corr")
            nc.sync.dma_start(corr[...], corrf[...])
            cidsf = pq.tile([4, BS], F32, tag="cidsf")
            mskf = pq.tile([4, BS], F32, tag="mskf")
            nc.vector.tensor_copy(cidsf[...], qc[0:4, :, 0])
            nc.vector.tensor_copy(mskf[...], qc[0:4, :, 1])
            vrawf = pq.tile([4, BS], F32, tag="vrawf")
            nc.vector.scalar_tensor_tensor(vrawf[...], corr[...], float(NUM_C),
                                           cidsf[...], op0=OP.mult, op1=OP.add)
            # masked -> zero pad column (500 in kt / 1000 in vt); swizzled
            # writes so the DRAM bounce below is contiguous on both sides
            k1 = pq.tile([4, BS], F32, tag="k1")
            v1 = pq.tile([4, BS], F32, tag="v1")
            nc.vector.scalar_tensor_tensor(k1[...], cidsf[...], -500.0,
                                           mskf[...], op0=OP.add, op1=OP.mult)
            nc.vector.scalar_tensor_tensor(v1[...], vrawf[...], -1000.0,
                                           mskf[...], op0=OP.add, op1=OP.mult)
            ki16 = pq.tile([4, BS], I16, tag="ki16")
            vi16 = pq.tile([4, BS], I16, tag="vi16")
            nc.vector.tensor_scalar_add(
                ki16[...].rearrange("j (b0 b1) -> j b1 b0", b0=16),
                k1[...], 500.0)
            nc.vector.tensor_scalar_add(
                vi16[...].rearrange("j (b0 b1) -> j b1 b0", b0=16),
                v1[...], 1000.0)

            # den = max(sum_j mask, 1); idb = broadcast(1/den)
            inv_bf = pq.tile([1, BS], BF16, tag="inv_bf")
            for c in range(4):
                sl = slice(c * 400, (c + 1) * 400)
                msum_ps = psA.tile([1, 400], F32, tag="mm1", bufs=2,
                                    name=f"msum{c}")
                nc.tensor.matmul(msum_ps[...], ones4[...], mskf[:, sl])
                den_c = pq.tile([1, 400], F32, tag="den", bufs=2,
                                name=f"den{c}")
                nc.vector.tensor_scalar_max(den_c[...], msum_ps[...], 1.0)
                nc.vector.reciprocal(inv_bf[:, sl], den_c[...])
            nc.gpsimd.partition_broadcast(idb[...], inv_bf[...])

            # ---- P3: rewrap indices via DRAM bounce ----
            # Flat index list is j-major: n = j*BS + bs. The wrapped layout
            # puts index n at [n % 16, n // 16]; write DRAM directly in
            # wrapped order (dram[(n%16)*400 + n//16]) so the SBUF load is
            # 8 contiguous replicas.
            for i16, wrap, nm in ((ki16, kwrap, "kb"), (vi16, vwrap, "vb")):
                bounce = dram.tile([4 * BS], I16, tag=f"bounce_{nm}",
                                   name=f"bounce_{nm}")
                bview = bounce[...].rearrange("(b0 x) -> b0 x", b0=16)
                for j in range(4):
                    nc.sync.dma_start(
                        bview[:, j * 100:(j + 1) * 100],
                        i16[j:j + 1, :].rearrange("j (b0 b1) -> j b0 b1",
                                                  b0=16))
                wsrc = bounce[...].rearrange("(p col) -> p col", p=16)
                for g in range(8):
                    nc.sync.dma_start(wrap[16 * g:16 * (g + 1), :], wsrc)

        with tc.tile_pool(name="pg", bufs=1) as pg:
            # ---- P4: embedding gathers (SBUF tables, j-major order) ----
            kg = pg.tile([DK, 4 * BS], F32, tag="kg")
            vg = pg.tile([DK, 4 * BS], F32, tag="vg")
            nc.gpsimd.ap_gather(kg[...].unsqueeze(2), kt[...].unsqueeze(2),
                                kwrap[...], channels=128, num_elems=KPAD,
                                d=1, num_idxs=4 * BS)
            nc.gpsimd.ap_gather(vg[...].unsqueeze(2), vt[...].unsqueeze(2),
                                vwrap[...], channels=128, num_elems=VPAD,
                                d=1, num_idxs=4 * BS)

            # ---- P5: j-sum (j-major blocks, contiguous adds) + mean ----
            for gsrc, bar, nm in ((kg, kbar, "k"), (vg, vbar, "v")):
                s01 = pg.tile([DK, BS], BF16, tag=f"{nm}s01", name=f"{nm}s01")
                s23 = pg.tile([DK, BS], BF16, tag=f"{nm}s23", name=f"{nm}s23")
                ssum = pg.tile([DK, BS], BF16, tag=f"{nm}ss", name=f"{nm}ss")
                nc.vector.tensor_add(s01[...], gsrc[:, 0:BS],
                                     gsrc[:, BS:2 * BS])
                nc.vector.tensor_add(s23[...], gsrc[:, 2 * BS:3 * BS],
                                     gsrc[:, 3 * BS:4 * BS])
                nc.vector.tensor_add(ssum[...], s01[...], s23[...])
                nc.vector.tensor_mul(bar[...], ssum[...], idb[...])

        # ---- P6: w = softmax(kbar^T @ Mk^T) per bs-chunk ----
        nc.gpsimd.memset(w_rows[...], 0.0)
        for c in range(NCH):
            p = min(128, BS - c * 128)
            sl = slice(c * 128, c * 128 + p)
            lg = psA.tile([128, SLOTS], F32, tag="mm2", bufs=4)
            nc.tensor.matmul(lg[:p, :], kbar[:, sl], mkt[...])
            mx = main.tile([128, 1], F32, tag="mx")
            sx = main.tile([128, 1], F32, tag="sx")
            rx = main.tile([128, 1], F32, tag="rx")
            ex = main.tile([128, SLOTS], F32, tag="ex")
            nc.vector.tensor_reduce(mx[:p, :], lg[:p, :],
                                    axis=mybir.AxisListType.X, op=OP.max,
                                    negate=True)
            nc.scalar.activation(ex[:p, :], lg[:p, :], AF.Exp,
                                 bias=mx[:p, :], scale=1.0,
                                 accum_out=sx[:p, :])
            nc.vector.reciprocal(rx[:p, :], sx[:p, :])
            nc.vector.tensor_scalar_mul(w_rows[:p, c, :], ex[:p, :], rx[:p, :])

        # reorder w into per-step rows: w_flat[t0, t1, n, b] = w[b, n, t]
        wdram = dram.tile([NCH * 128 * SLOTS], BF16, tag="wdram")
        nc.sync.dma_start(
            wdram[...].rearrange("(c p n) -> p c n", p=128, n=SLOTS),
            w_rows[...])
        for k3 in range(3):
            cnt = (S - k3 + 2) // 3
            src3 = wdram[k3 * NB:k3 * NB + cnt * 3 * NB] \
                .rearrange("(u j bn) -> u j bn", j=3, bn=NB)[:, 0, :]
            nc.sync.dma_start(w32[32 * k3:32 * k3 + 1, 0:cnt, :], src3)

        # ---- P7: e/a ----
        for c in range(4):
            sl = slice(c * 400, (c + 1) * 400)
            ep = psA.tile([DK, 400], F32, tag="mm2", bufs=4)
            nc.tensor.matmul(ep[...], ewt[...], vbar[:, sl])
            nc.scalar.activation(e_all[:, sl], ep[...], AF.Sigmoid,
                                 bias=eb[...], scale=1.0)
            ap_ = psA.tile([DK, 400], F32, tag="mm2", bufs=4)
            nc.tensor.matmul(ap_[...], awt[...], vbar[:, sl])
            nc.scalar.activation(a_all[:, sl], ap_[...], AF.Tanh,
                                 bias=ab[...], scale=1.0)

        psA_stack.close()

        # ---- P8: recurrence ----
        # w_t is pre-broadcast across partitions in 4-step groups:
        # PE rank-0 matmul (ones[1,128]^T x w_row[1,400]) into PSUM banks,
        # then one bulk ACT copy -> bf16 SBUF (ping-pong pair of buffers).
        GRP = 4
        NGRP = S // GRP
        ones128 = const.tile([128, DK], BF16, tag="ones128")
        nc.vector.memset(ones128[...], 1.0)
        WRING = 3
        wbuf = [main.tile([DK, GRP * NB], BF16, tag=f"wbuf{i}",
                          name=f"wbuf{i}") for i in range(WRING)]

        mv = [main.tile([DK, NB], BF16, tag=f"mv{i}", name=f"mv{i}")
              for i in range(2)]
        nc.sync.dma_start(mv[0][...], mv0_d[...])
        p0 = [main.tile([DK, NB], BF16, tag=f"p0{i}", name=f"p0{i}")
               for i in range(4)]
        t1_ = [main.tile([DK, NB], BF16, tag=f"t1{i}", name=f"t1{i}")
               for i in range(2)]
        t2_ = [main.tile([DK, NB], BF16, tag=f"t2{i}", name=f"t2{i}")
               for i in range(4)]
        t3_ = [main.tile([DK, NB], BF16, tag=f"t3{i}", name=f"t3{i}")
               for i in range(2)]

        with ExitStack() as rstk:
            psR = rstk.enter_context(
                tc.tile_pool(name="psR", bufs=1, space="PSUM"))

            def emit_wb_group(g):
                # w rows -> all partitions (PE) -> bf16 SBUF (ACT)
                wbps = psR.tile([DK, GRP * 512], F32, tag="wbps", bufs=2,
                                name=f"wbps{g}")
                for s in range(GRP):
                    t = g * GRP + s
                    al = 32 * (t % 3)
                    nc.tensor.matmul(
                        wbps[:, 512 * s:512 * s + NB],
                        ones128[al:al + 1, :],
                        w32[al:al + 1, t // 3, :])
                nc.scalar.activation(
                    wbuf[g % WRING][...].rearrange("p (s x) -> p s x", s=GRP),
                    wbps[...].rearrange("p (s x) -> p s x", x=512)[:, :, 0:NB],
                    AF.Copy)

            emit_wb_group(0)
            emit_wb_group(1)
            for t in range(S):
                g = t // GRP
                if t % GRP == 0 and g + 2 < NGRP:
                    emit_wb_group(g + 2)
                k = t % 2
                cur, nxt = mv[k], mv[1 - k]
                p0k, t1k, t2k, t3k = p0[t % 4], t1_[k], t2_[t % 4], t3_[k]
                off = (t % GRP) * NB
                wbk = wbuf[g % WRING][:, off:off + NB]
                ev = e_all[:, t * BL:(t + 1) * BL].unsqueeze(1) \
                    .broadcast_to([DK, SLOTS, BL])
                av = a_all[:, t * BL:(t + 1) * BL].unsqueeze(1) \
                    .broadcast_to([DK, SLOTS, BL])
                p03 = p0k[...].rearrange("p (n b) -> p n b", b=BL)
                t13 = t1k[...].rearrange("p (n b) -> p n b", b=BL)
                wb3 = wbk.rearrange("p (n b) -> p n b", b=BL)
                t23 = t2k[...].rearrange("p (n b) -> p n b", b=BL)
                # p0 = Mv*w ; t2 = w*a (independent of p0)
                nc.vector.tensor_tensor(p0k[...], cur[...], wbk, OP.mult)
                nc.vector.tensor_tensor(t23, wb3, av, OP.mult)
                # t1 = p0*e ; t3 = t2 - t1 ; Mv' = Mv + t3
                nc.vector.tensor_tensor(t13, p03, ev, OP.mult)
                # read path (off critical chain)
                nc.vector.tensor_reduce(
                    reads_bf[:, t * BL:(t + 1) * BL],
                    p0k[...].rearrange("p (n b) -> p b n", b=BL),
                    axis=mybir.AxisListType.X, op=OP.add)
                nc.vector.tensor_sub(t3k[...], t2k[...], t1k[...])
                nc.vector.tensor_add(nxt[...], cur[...], t3k[...])

        # ---- P9: output head ----
        psB_stack = ExitStack()
        psB = psB_stack.enter_context(
            tc.tile_pool(name="psB", bufs=1, space="PSUM"))
        for c in range(4):
            sl = slice(c * 400, (c + 1) * 400)
            fp = psB.tile([DK, 400], F32, tag="mm2", bufs=4)
            nc.tensor.matmul(fp[...], fw1t[...], reads_bf[:, sl],
                             start=True, stop=False)
            nc.tensor.matmul(fp[...], fw2t[...], kbar[:, sl],
                             start=False, stop=True)
            nc.scalar.activation(f_all[:, sl], fp[...], AF.Tanh,
                                 bias=fb[...], scale=1.0)
        for c in range(4):
            sl = slice(c * 400, (c + 1) * 400)
            pp = psB.tile([1, 400], F32, tag="mm1", bufs=2)
            nc.tensor.matmul(pp[...], pwt[...], f_all[:, sl])
            nc.scalar.activation(out_sb[:, sl], pp[...], AF.Sigmoid,
                                 bias=pb[...], scale=1.0)
        nc.sync.dma_start(out_d[...], out_sb[...])
        psB_stack.close()

    nc.finalize()
    return nc


def _host_inputs(inputs):
    """Build per-core + replicated DRAM inputs from the full problem inputs."""
    bf = ml_dtypes.bfloat16
    qs = np.asarray(inputs["question_seq"]).astype(np.int64)
    cs = np.asarray(inputs["correctness_seq"]).astype(np.int64)
    q2c = np.asarray(inputs["q2c_table"]).astype(np.int32)
    q2m = np.asarray(inputs["q2c_mask"]).astype(np.int32)
    ke = np.asarray(inputs["key_embed"], np.float32)
    ve = np.asarray(inputs["value_embed"], np.float32)
    mk = np.asarray(inputs["Mk"], np.float32)
    mv0 = np.asarray(inputs["Mv0"], np.float32)
    fw = np.asarray(inputs["f_W"], np.float32)
    fb = np.asarray(inputs["f_b"], np.float32)
    ew = np.asarray(inputs["e_W"], np.float32)
    eb = np.asarray(inputs["e_b"], np.float32)
    aw = np.asarray(inputs["a_W"], np.float32)
    ab = np.asarray(inputs["a_b"], np.float32)
    pw = np.asarray(inputs["p_W"], np.float32)
    pb = np.asarray(inputs["p_b"], np.float32)

    rep = {
        "q2c_comb": np.concatenate(
            [np.stack([q2c.T, q2m.T], 2).reshape(4, 2 * NUM_Q),
             np.zeros((12, 2 * NUM_Q), np.int64)], 0
        ).astype(np.int16),
        "kt": np.concatenate([ke.T, np.zeros((DK, KPAD - NUM_C), np.float32)],
                             1).astype(np.float32),
        "vt": np.concatenate([ve.T, np.zeros((DK, VPAD - 2 * NUM_C),
                                             np.float32)], 1).astype(np.float32),

        "mkt": mk.T.astype(bf),
        "ewt": ew.T.astype(bf),
        "awt": aw.T.astype(bf),
        "fw1t": fw[:, :DK].T.astype(bf),
        "fw2t": fw[:, DK:].T.astype(bf),
        "pwt": pw.T.astype(bf),
        "eb": eb.reshape(DK, 1).astype(np.float32),
        "ab": ab.reshape(DK, 1).astype(np.float32),
        "fb": fb.reshape(DK, 1).astype(np.float32),
        "pb": pb.reshape(1, 1).astype(np.float32),
        "mv0r": np.repeat(mv0.T, BL, axis=1).astype(bf),
    }
    in_maps = []
    for core in range(NCORES):
        q_flat = qs[core * BL:(core + 1) * BL].T.reshape(-1)   # t-major
        c_flat = cs[core * BL:(core + 1) * BL].T.reshape(-1)
        m = dict(rep)
        m["qseq_w"] = np.ascontiguousarray(
            q_flat.reshape(BS // 16, 16).T).astype(np.int16)
        m["corrf"] = np.broadcast_to(c_flat.astype(np.float32),
                                     (4, BS)).copy()
        in_maps.append(m)
    return in_maps


def kernel(**inputs):
    global _PROG
    if _PROG is None:
        _PROG = _build_program()
    in_maps = _host_inputs(inputs)
    res = run_bass_kernel_spmd(_PROG, in_maps, core_ids=list(range(NCORES)))
    out = np.zeros((B, S), np.float32)
    for core in range(NCORES):
        o = res.results[core]["out"].reshape(S, BL)
        out[core * BL:(core + 1) * BL] = o.T
    return out



# revision 17
# speedup vs baseline: 1.8280x; 1.8280x over previous
# DKVMN Trainium2 Bass kernel (v2).
#
# Sharding: data-parallel over batch across 8 NeuronCores (8 sequences each);
# embedding tables and all parameters replicated.
#
# Per-core program (bs = t*8 + b, "t-major", BS=1600):
#   P1  q2c_table/q2c_mask rows gathered by question id (gpsimd ap_gather).
#   P2  index math on DVE: v_idx = cid + 500*correct; masked entries are
#       redirected to an all-zero pad row (500 in key table / 1000 in value).
#   P3  indices flattened to [1, 4*BS] fp16 via DRAM bounce, then
#       partition-broadcast to all 128 partitions.
#   P4  one-hot COUNT matrices built by iota-compare on DVE (fp16, 4x mode);
#       value-table counts derived by correctness masking.  Embedding
#       "gathers" then become PE matmuls: kbar^T = sum_c table[c,:]*count[c,bs]
#       with the masked mean folded in afterwards via 1/den broadcast.
#   P6  w = softmax(kbar^T Mk^T), batched: one PE pass into PSUM, one exp,
#       tree-sum over slots, one reciprocal, one scaled copy.
#   P7  e/a = sigmoid/tanh(vbar^T W^T + b) (PE + ACT), written time-inner
#       ([d, b, t]) for the recurrence.
#   P8  recurrence Mv_t = Mv_{t-1} * (1 - w e^T) + w a^T evaluated as an
#       affine prefix scan (tensor_tensor_scan) over 10 chunks of 20 steps.
#       Per chunk: PE broadcasts w rows across partitions (rank-1 matmuls into
#       PSUM), ACT copies them to bf16 SBUF in [d, n, b, t] layout, DVE builds
#       A = 1 - w*e and B = w*a (+ carry slot) with 2x/4x-mode ops, one scan
#       instruction advances all 400 chains 20 steps, then reads are a batched
#       multiply + add-tree over slots.
#   P9  f = tanh([reads, kbar] f_W^T + f_b); out = sigmoid(f p_W^T + p_b).
import sys

for _p in ("/opt/trn_rl_repo", "/root/.axon_site/_ro/trn_rl_repo"):
    if _p not in sys.path:
        sys.path.append(_p)

from contextlib import ExitStack

import numpy as np
import ml_dtypes

import concourse.bass as bass
import concourse.bacc as bacc
import concourse.mybir as mybir
from concourse.bass_utils import run_bass_kernel_spmd
from concourse.tile import TileContext

F32 = mybir.dt.float32
BF16 = mybir.dt.bfloat16
FP16 = mybir.dt.float16
I32 = mybir.dt.int32
I16 = mybir.dt.int16
AF = mybir.ActivationFunctionType
OP = mybir.AluOpType

B, S, DK, SLOTS = 64, 200, 128, 50
NUM_Q, NUM_C, MAXC = 10000, 500, 4
NCORES = 8
BL = B // NCORES          # 8 sequences per core
BS = BL * S               # 1600 (bs = t*BL + b)
NB = SLOTS * BL           # 400 state columns (n-major, b-inner)
KCH = 4                   # key table: 4 chunks of 128 rows (512 padded)
VCH = 8                   # value table: 8 chunks (1024 padded)
NCH = (BS + 127) // 128   # 13 bs-chunks for softmax
TCH = 20                  # recurrence chunk length (steps)
NCHK = S // TCH           # 10 chunks
CJ = TCH + 1              # scan slots per chain (slot 0 = carry/reset)

_PROG = None  # cached compiled program


def _build_program():
    nc = bacc.Bacc("TRN2", target_bir_lowering=False, debug=False,
                   num_devices=NCORES)

    def din(name, shape, dt):
        return nc.dram_tensor(name, shape, dt, kind="ExternalInput")

    qseq_w = din("qseq_w", [16, BS // 16], I16)
    corrf = din("corrf", [4, BS], F32)
    q2c_comb = din("q2c_comb", [16, 2 * NUM_Q], I16)
    ket_d = din("ket", [KCH * 128, DK], FP16)
    vet_d = din("vet", [VCH * 128, DK], FP16)
    mkt_d = din("mkt", [DK, SLOTS], BF16)
    ewt_d = din("ewt", [DK, DK], BF16)
    awt_d = din("awt", [DK, DK], BF16)
    fw1t_d = din("fw1t", [DK, DK], BF16)
    fw2t_d = din("fw2t", [DK, DK], BF16)
    pwt_d = din("pwt", [DK, 1], BF16)
    eb_d = din("eb", [DK, 1], F32)
    ab_d = din("ab", [DK, 1], F32)
    fb_d = din("fb", [DK, 1], F32)
    pb_d = din("pb", [1, 1], F32)
    mv0_d = din("mv0r", [DK, NB], BF16)
    out_d = nc.dram_tensor("out", [1, BS], F32, kind="ExternalOutput")

    with ExitStack() as ctx:
        ctx.enter_context(
            nc.allow_low_precision("bf16 state; rel-err budget 2e-2"))
        tc = ctx.enter_context(TileContext(nc))
        const = ctx.enter_context(tc.tile_pool(name="const", bufs=1))
        main = ctx.enter_context(tc.tile_pool(name="main", bufs=1))
        dram = ctx.enter_context(tc.tile_pool(name="dram", bufs=1,
                                              space="DRAM"))

        # ---- persistent tiles ----
        kbar = main.tile([DK, BS], BF16, tag="kbar")
        e_tin = main.tile([DK, BS], BF16, tag="e_tin")    # [d, b, t]
        a_tin = main.tile([DK, BS], BF16, tag="a_tin")    # [d, b, t]
        idb = main.tile([DK, BS], BF16, tag="idb")
        w_rows = main.tile([128, NCH, SLOTS], BF16, tag="w_rows")
        reads_bs = main.tile([DK, BS], BF16, tag="reads_bs")
        f_all = main.tile([DK, BS], BF16, tag="f_all")
        out_sb = main.tile([1, BS], F32, tag="out_sb")

        # ---- load params (const pool, alive whole kernel) ----
        kes = const.tile([128, KCH, DK], FP16, tag="kes")
        ves = const.tile([128, VCH, DK], FP16, tag="ves")
        mkt = const.tile([DK, SLOTS], BF16, tag="mkt")
        ewt = const.tile([DK, DK], BF16, tag="ewt")
        awt = const.tile([DK, DK], BF16, tag="awt")
        fw1t = const.tile([DK, DK], BF16, tag="fw1t")
        fw2t = const.tile([DK, DK], BF16, tag="fw2t")
        pwt = const.tile([DK, 1], BF16, tag="pwt")
        eb = const.tile([DK, 1], F32, tag="eb")
        ab = const.tile([DK, 1], F32, tag="ab")
        fb = const.tile([DK, 1], F32, tag="fb")
        pb = const.tile([1, 1], F32, tag="pb")
        ones4 = const.tile([4, 1], F32, tag="ones4")
        ones128 = const.tile([128, DK], BF16, tag="ones128")
        iof = const.tile([128, 1], F32, tag="iof")
        nc.vector.memset(ones4[...], 1.0)
        nc.vector.memset(ones128[...], 1.0)
        ioi = const.tile([128, 1], I32, tag="ioi")
        nc.gpsimd.iota(ioi[...], pattern=[[0, 1]], base=0,
                       channel_multiplier=1)
        nc.vector.tensor_copy(iof[...], ioi[...])
        nc.sync.dma_start(kes[...],
                          ket_d[...].rearrange("(c p) d -> p c d", p=128))
        nc.sync.dma_start(ves[...],
                          vet_d[...].rearrange("(c p) d -> p c d", p=128))
        for tile_, dt_ in ((mkt, mkt_d), (ewt, ewt_d), (awt, awt_d),
                           (fw1t, fw1t_d), (fw2t, fw2t_d), (pwt, pwt_d),
                           (eb, eb_d), (ab, ab_d), (fb, fb_d), (pb, pb_d)):
            nc.sync.dma_start(tile_[...], dt_[...])

        psA_stack = ExitStack()
        psA = psA_stack.enter_context(
            tc.tile_pool(name="psA", bufs=1, space="PSUM"))

        bounce_k = dram.tile([4 * BS], FP16, tag="bounce_kb",
                             name="bounce_kb")
        bounce_v = dram.tile([4 * BS], FP16, tag="bounce_vb",
                             name="bounce_vb")
        if True:
            with tc.tile_pool(name="pfA", bufs=1) as pfA:
                # ---- P1: gather cids/mask rows by question id ----
                q2c_t = pfA.tile([16, NUM_Q, 2], I16, tag="q2c")
                qw = pfA.tile([16, BS // 16], I16, tag="qw")
                nc.sync.dma_start(q2c_t[...], q2c_comb[...])
                nc.sync.dma_start(qw[...], qseq_w[...])
                qc = pfA.tile([16, BS, 2], I16, tag="qc")
                nc.gpsimd.ap_gather(qc[...], q2c_t[...], qw[...], channels=16,
                                    num_elems=NUM_Q, d=2, num_idxs=BS)

                # ---- P2: index math (f32, exact) ----
                corr = pfA.tile([4, BS], F32, tag="corr")
                nc.sync.dma_start(corr[...], corrf[...])
                cidsf = pfA.tile([4, BS], F32, tag="cidsf")
                mskf = pfA.tile([4, BS], F32, tag="mskf")
                nc.vector.tensor_copy(cidsf[...], qc[0:4, :, 0])
                nc.vector.tensor_copy(mskf[...], qc[0:4, :, 1])
                vrawf = pfA.tile([4, BS], F32, tag="vrawf")
                nc.vector.scalar_tensor_tensor(vrawf[...], corr[...],
                                               float(NUM_C), cidsf[...],
                                               op0=OP.mult, op1=OP.add)
                # masked -> zero pad row (500 in key / 1000 in value table)
                k1 = pfA.tile([4, BS], F32, tag="k1")
                v1 = pfA.tile([4, BS], F32, tag="v1")
                nc.vector.scalar_tensor_tensor(k1[...], cidsf[...], -500.0,
                                               mskf[...], op0=OP.add,
                                               op1=OP.mult)
                nc.vector.scalar_tensor_tensor(v1[...], vrawf[...], -1000.0,
                                               mskf[...], op0=OP.add,
                                               op1=OP.mult)
                kh = pfA.tile([4, BS], FP16, tag="kh")
                vh = pfA.tile([4, BS], FP16, tag="vh")
                nc.vector.tensor_scalar_add(kh[...], k1[...], 500.0)
                nc.vector.tensor_scalar_add(vh[...], v1[...], 1000.0)
                nc.sync.dma_start(
                    bounce_k[...].rearrange("(j x) -> j x", j=4), kh[...])
                nc.sync.dma_start(
                    bounce_v[...].rearrange("(j x) -> j x", j=4), vh[...])

                # den = max(sum_j mask, 1); idb = broadcast(1/den)
                inv_bf = pfA.tile([1, BS], BF16, tag="inv_bf")
                for c in range(4):
                    sl = slice(c * 400, (c + 1) * 400)
                    msum_ps = psA.tile([1, 400], F32, tag="mm1", bufs=2,
                                       name=f"msum{c}")
                    nc.tensor.matmul(msum_ps[...], ones4[...], mskf[:, sl])
                    den_c = pfA.tile([1, 400], F32, tag="den", bufs=2,
                                     name=f"den{c}")
                    nc.vector.tensor_scalar_max(den_c[...], msum_ps[...], 1.0)
                    nc.vector.reciprocal(inv_bf[:, sl], den_c[...])
                nc.gpsimd.partition_broadcast(idb[...], inv_bf[...])

            pf_stack = ExitStack()
            pf = pf_stack.enter_context(tc.tile_pool(name="pf", bufs=1))
            # ---- P3: reload flattened indices, broadcast to 128 parts ----
            kfl = pf.tile([1, 4 * BS], FP16, tag="kfl")
            vfl = pf.tile([1, 4 * BS], FP16, tag="vfl")
            nc.sync.dma_start(kfl[...],
                              bounce_k[...].rearrange("(o x) -> o x", o=1))
            nc.sync.dma_start(vfl[...],
                              bounce_v[...].rearrange("(o x) -> o x", o=1))
            kbi = pf.tile([128, 4 * BS], FP16, tag="kbi")
            vbi = pf.tile([128, 4 * BS], FP16, tag="vbi")
            nc.gpsimd.partition_broadcast(kbi[...], kfl[...])
            nc.gpsimd.partition_broadcast(vbi[...], vfl[...])

            # ---- P4: count matrices by iota-compare; PE "gathers" ----
            cntk = pf.tile([128, KCH, BS], FP16, tag="cntk")
            wvt = pf.tile([128, VCH, BS], FP16, tag="wvt")
            isq = pf.tile([128, 4, BS], FP16, tag="isq")
            s01 = pf.tile([128, BS], FP16, tag="s01")
            s23 = pf.tile([128, BS], FP16, tag="s23")
            iotc = pf.tile([128, 1], F32, tag="iotc")
            kbi3 = kbi[...].rearrange("p (j x) -> p j x", j=4)
            for c in range(KCH):
                nc.vector.tensor_scalar_add(iotc[...], iof[...],
                                            float(128 * c))
                nc.vector.tensor_scalar(isq[...], kbi3, iotc[...], None,
                                        op0=OP.is_equal)
                nc.vector.tensor_add(s01[...], isq[:, 0, :], isq[:, 1, :])
                nc.vector.tensor_add(s23[...], isq[:, 2, :], isq[:, 3, :])
                nc.vector.tensor_add(cntk[:, c, :], s01[...], s23[...])
            vbi3 = vbi[...].rearrange("p (j x) -> p j x", j=4)
            for c in range(VCH):
                nc.vector.tensor_scalar_add(iotc[...], iof[...],
                                            float(128 * c))
                nc.vector.tensor_scalar(isq[...], vbi3, iotc[...], None,
                                        op0=OP.is_equal)
                nc.vector.tensor_add(s01[...], isq[:, 0, :], isq[:, 1, :])
                nc.vector.tensor_add(s23[...], isq[:, 2, :], isq[:, 3, :])
                nc.vector.tensor_add(wvt[:, c, :], s01[...], s23[...])

            vbar = pf.tile([DK, BS], BF16, tag="vbar")
            for s in range(4):
                sl = slice(s * 400, (s + 1) * 400)
                kb_ps = psA.tile([DK, 400], F32, tag="mm2", bufs=4)
                for c in range(KCH):
                    nc.tensor.matmul(kb_ps[...], kes[:, c, :],
                                     cntk[:, c, sl], start=(c == 0),
                                     stop=(c == KCH - 1))
                nc.vector.tensor_mul(kbar[:, sl], kb_ps[...], idb[:, sl])
                vb_ps = psA.tile([DK, 400], F32, tag="mm2", bufs=4)
                for c in range(VCH):
                    nc.tensor.matmul(vb_ps[...], ves[:, c, :],
                                     wvt[:, c, sl], start=(c == 0),
                                     stop=(c == VCH - 1))
                nc.vector.tensor_mul(vbar[:, sl], vb_ps[...], idb[:, sl])

            # ---- P6: w = softmax(kbar^T @ Mk^T), batched ----
            # 64-slot padding keeps every matmul output inside one PSUM bank
            lg = psA.tile([128, NCH, 64], F32, tag="mm3", bufs=1)
            for c in range(NCH):
                p = min(128, BS - c * 128)
                nc.tensor.matmul(lg[:p, c, 0:SLOTS],
                                 kbar[:, c * 128:c * 128 + p], mkt[...])
            ex = pf.tile([128, NCH, SLOTS], F32, tag="ex")
            nc.scalar.activation(ex[...], lg[:, :, 0:SLOTS], AF.Exp)
            # tree-sum over slots (50 = 25+25 -> 12+12(+1) -> 6+6 -> 3+3 ...)
            t25 = pf.tile([128, NCH, 25], F32, tag="t25")
            t12 = pf.tile([128, NCH, 12], F32, tag="t12")
            t6 = pf.tile([128, NCH, 6], F32, tag="t6")
            t3 = pf.tile([128, NCH, 3], F32, tag="t3")
            sx = pf.tile([128, NCH, 1], F32, tag="sx")
            rx = pf.tile([128, NCH], F32, tag="rx")
            nc.vector.tensor_add(t25[...], ex[:, :, 0:25], ex[:, :, 25:50])
            nc.vector.tensor_add(t12[...], t25[:, :, 0:12], t25[:, :, 12:24])
            nc.vector.tensor_add(t6[...], t12[:, :, 0:6], t12[:, :, 6:12])
            nc.vector.tensor_add(t3[...], t6[:, :, 0:3], t6[:, :, 3:6])
            nc.vector.tensor_add(sx[...], t3[:, :, 0:1], t3[:, :, 1:2])
            nc.vector.tensor_add(sx[...], sx[...], t3[:, :, 2:3])
            nc.vector.tensor_add(sx[...], sx[...], t25[:, :, 24:25])
            nc.vector.reciprocal(rx[...], sx[:, :, 0])
            nc.vector.tensor_tensor(
                w_rows[...], ex[...],
                rx[...].unsqueeze(2).broadcast_to([128, NCH, SLOTS]),
                OP.mult)

            # reorder w into per-step rows via DRAM bounce (loaded in P8)
            wdram = dram.tile([NCH * 128 * SLOTS], BF16, tag="wdram")
            nc.sync.dma_start(
                wdram[...].rearrange("(c p n) -> p c n", p=128, n=SLOTS),
                w_rows[...])

            # ---- P7: e/a, written time-inner [d, b, t] ----
            for c in range(4):
                sl = slice(c * 400, (c + 1) * 400)
                tv = slice(c * 50, (c + 1) * 50)
                ep = psA.tile([DK, 400], F32, tag="mm2", bufs=4)
                nc.tensor.matmul(ep[...], ewt[...], vbar[:, sl])
                nc.scalar.activation(
                    e_tin[...].rearrange("p (b t) -> p t b", b=BL)[:, tv, :],
                    ep[...].rearrange("p (t b) -> p t b", b=BL),
                    AF.Sigmoid, bias=eb[...], scale=1.0)
                ap_ = psA.tile([DK, 400], F32, tag="mm2", bufs=4)
                nc.tensor.matmul(ap_[...], awt[...], vbar[:, sl])
                nc.scalar.activation(
                    a_tin[...].rearrange("p (b t) -> p t b", b=BL)[:, tv, :],
                    ap_[...].rearrange("p (t b) -> p t b", b=BL),
                    AF.Tanh, bias=ab[...], scale=1.0)

            pf_stack.close()

        psA_stack.close()

        # ---- P8: recurrence as chunked affine prefix scan ----
        # Chains: c = n*BL + b (n-major), slots j=0..TCH; slot 0 is the
        # carry-in (A=0, B=Mv at chunk start), slot j = step t0+j.
        with ExitStack() as rstk:
            pr = rstk.enter_context(tc.tile_pool(name="pr", bufs=1))
            psW = rstk.enter_context(
                tc.tile_pool(name="psW", bufs=1, space="PSUM"))

            w32 = pr.tile([128, (S + 2) // 3, NB], BF16, tag="w32")
            for k3 in range(3):
                cnt = (S - k3 + 2) // 3
                src3 = wdram[k3 * NB:k3 * NB + cnt * 3 * NB] \
                    .rearrange("(u j bn) -> u j bn", j=3, bn=NB)[:, 0, :]
                nc.sync.dma_start(w32[32 * k3:32 * k3 + 1, 0:cnt, :], src3)

            wsb = [pr.tile([128, TCH * NB], BF16, tag=f"wsb{i}",
                           name=f"wsb{i}") for i in range(2)]
            A_t = pr.tile([128, NB * CJ], BF16, tag="A_t")
            B_t = pr.tile([128, NB * CJ], BF16, tag="B_t")
            st = pr.tile([128, NB * CJ], BF16, tag="st")
            scr = pr.tile([128, TCH * NB], BF16, tag="scr")
            p0 = pr.tile([128, TCH * NB], BF16, tag="p0")
            # reduce-tree scratch aliased into scr (we-product is dead by
            # the time the tree runs)
            q1 = scr[:, 0:25 * BL * TCH]
            q2 = scr[:, 4000:4000 + 12 * BL * TCH]
            q3 = scr[:, 5920:5920 + 6 * BL * TCH]
            q4 = scr[:, 6880:6880 + 3 * BL * TCH]
            q5 = scr[:, 7360:7360 + BL * TCH]
            mv0s = pr.tile([DK, NB], BF16, tag="mv0s")

            nc.vector.memset(A_t[...], 0.0)  # slot 0 stays 0 forever
            nc.sync.dma_start(mv0s[...], mv0_d[...])
            Bv4 = B_t[...].rearrange("p (n b j) -> p n b j", n=SLOTS, b=BL)
            Av4 = A_t[...].rearrange("p (n b j) -> p n b j", n=SLOTS, b=BL)
            st4 = st[...].rearrange("p (n b j) -> p n b j", n=SLOTS, b=BL)
            # initial carry: B[:, c, 0] = Mv0[c]
            nc.vector.tensor_copy(
                Bv4[:, :, :, 0],
                mv0s[...].rearrange("p (n b) -> p n b", b=BL))

            e3 = e_tin[...].rearrange("p (b t) -> p b t", b=BL)
            a3 = a_tin[...].rearrange("p (b t) -> p b t", b=BL)
            r3 = reads_bs[...].rearrange("p (t b) -> p b t", b=BL)

            for k in range(NCHK):
                wk = wsb[k % 2]
                wk4 = wk[...].rearrange("p (n b t) -> p n b t", n=SLOTS, b=BL)
                wkT = wk[...].rearrange("p (n b t) -> p t n b", n=SLOTS, b=BL)
                # PE: broadcast w rows for 20 steps (5 PSUM groups of 4)
                for g in range(TCH // 4):
                    wbps = psW.tile([128, 4 * 512], F32, tag="wbps", bufs=2,
                                    name=f"wbps{k}_{g}")
                    for s4 in range(4):
                        t = k * TCH + g * 4 + s4       # step index 0..199
                        al = 32 * (t % 3)
                        nc.tensor.matmul(
                            wbps[:, 512 * s4:512 * s4 + NB],
                            ones128[al:al + 1, :],
                            w32[al:al + 1, t // 3, :])
                    nc.scalar.activation(
                        wkT[:, g * 4:(g + 1) * 4, :, :],
                        wbps[...].rearrange(
                            "p (s x) -> p s x", x=512)[:, :, 0:NB]
                        .rearrange("p s (n b) -> p s n b", b=BL),
                        AF.Copy)

                tv = slice(k * TCH, (k + 1) * TCH)
                eb4 = e3[:, :, tv].unsqueeze(1).broadcast_to(
                    [128, SLOTS, BL, TCH])
                ab4 = a3[:, :, tv].unsqueeze(1).broadcast_to(
                    [128, SLOTS, BL, TCH])
                scr4 = scr[...].rearrange("p (n b t) -> p n b t",
                                          n=SLOTS, b=BL)
                # we = w*e ; A = 1 - we ; B = w*a
                nc.vector.tensor_tensor(scr4, wk4, eb4, OP.mult)
                nc.vector.tensor_scalar(Av4[:, :, :, 1:CJ], scr4, -1.0, 1.0,
                                        op0=OP.mult, op1=OP.add)
                nc.vector.tensor_tensor(Bv4[:, :, :, 1:CJ], wk4, ab4,
                                        OP.mult)
                if k > 0:
                    # carry: B[:, c, 0] = Mv at end of previous chunk
                    nc.vector.tensor_copy(Bv4[:, :, :, 0],
                                          st4[:, :, :, TCH])
                # advance all 400 chains TCH steps
                nc.vector.tensor_tensor_scan(st[...], A_t[...], B_t[...],
                                             0.0, op0=OP.mult, op1=OP.add)
                # reads: p0 = w_t * Mv_{t-1}, then add-tree over n
                p04 = p0[...].rearrange("p (n b t) -> p n b t",
                                        n=SLOTS, b=BL)
                nc.vector.tensor_tensor(p04, st4[:, :, :, 0:TCH], wk4,
                                        OP.mult)
                q14 = q1.rearrange("p (n b t) -> p n b t", n=25, b=BL)
                q24 = q2.rearrange("p (n b t) -> p n b t", n=12, b=BL)
                q34 = q3.rearrange("p (n b t) -> p n b t", n=6, b=BL)
                q44 = q4.rearrange("p (n b t) -> p n b t", n=3, b=BL)
                q54 = q5.rearrange("p (b t) -> p b t", b=BL)
                nc.vector.tensor_add(q14, p04[:, 0:25], p04[:, 25:50])
                nc.vector.tensor_add(q24, q14[:, 0:12], q14[:, 12:24])
                nc.vector.tensor_add(q34, q24[:, 0:6], q24[:, 6:12])
                nc.vector.tensor_add(q44, q34[:, 0:3], q34[:, 3:6])
                nc.vector.tensor_add(q54, q44[:, 0, :, :], q44[:, 1, :, :])
                nc.vector.tensor_add(q54, q54, q44[:, 2, :, :])
                nc.vector.tensor_add(r3[:, :, tv], q54, q14[:, 24, :, :])

        # ---- P9: output head ----
        psB_stack = ExitStack()
        psB = psB_stack.enter_context(
            tc.tile_pool(name="psB", bufs=1, space="PSUM"))
        for c in range(4):
            sl = slice(c * 400, (c + 1) * 400)
            fp = psB.tile([DK, 400], F32, tag="mm2", bufs=4)
            nc.tensor.matmul(fp[...], fw1t[...], reads_bs[:, sl],
                             start=True, stop=False)
            nc.tensor.matmul(fp[...], fw2t[...], kbar[:, sl],
                             start=False, stop=True)
            nc.scalar.activation(f_all[:, sl], fp[...], AF.Tanh,
                                 bias=fb[...], scale=1.0)
        for c in range(4):
            sl = slice(c * 400, (c + 1) * 400)
            pp = psB.tile([1, 400], F32, tag="mm1", bufs=2)
            nc.tensor.matmul(pp[...], pwt[...], f_all[:, sl])
            nc.scalar.activation(out_sb[:, sl], pp[...], AF.Sigmoid,
                                 bias=pb[...], scale=1.0)
        nc.sync.dma_start(out_d[...], out_sb[...])
        psB_stack.close()

    nc.finalize()
    return nc


def _host_inputs(inputs):
    """Build per-core + replicated DRAM inputs from the full problem inputs."""
    bf = ml_dtypes.bfloat16
    qs = np.asarray(inputs["question_seq"]).astype(np.int64)
    cs = np.asarray(inputs["correctness_seq"]).astype(np.int64)
    q2c = np.asarray(inputs["q2c_table"]).astype(np.int32)
    q2m = np.asarray(inputs["q2c_mask"]).astype(np.int32)
    ke = np.asarray(inputs["key_embed"], np.float32)
    ve = np.asarray(inputs["value_embed"], np.float32)
    mk = np.asarray(inputs["Mk"], np.float32)
    mv0 = np.asarray(inputs["Mv0"], np.float32)
    fw = np.asarray(inputs["f_W"], np.float32)
    fb = np.asarray(inputs["f_b"], np.float32)
    ew = np.asarray(inputs["e_W"], np.float32)
    eb = np.asarray(inputs["e_b"], np.float32)
    aw = np.asarray(inputs["a_W"], np.float32)
    ab = np.asarray(inputs["a_b"], np.float32)
    pw = np.asarray(inputs["p_W"], np.float32)
    pb = np.asarray(inputs["p_b"], np.float32)

    kep = np.zeros((KCH * 128, DK), np.float16)
    kep[:NUM_C] = ke.astype(np.float16)
    vep = np.zeros((VCH * 128, DK), np.float16)
    vep[:2 * NUM_C] = ve.astype(np.float16)

    rep = {
        "q2c_comb": np.concatenate(
            [np.stack([q2c.T, q2m.T], 2).reshape(4, 2 * NUM_Q),
             np.zeros((12, 2 * NUM_Q), np.int64)], 0
        ).astype(np.int16),
        "ket": kep,
        "vet": vep,
        "mkt": mk.T.astype(bf),
        "ewt": ew.T.astype(bf),
        "awt": aw.T.astype(bf),
        "fw1t": fw[:, :DK].T.astype(bf),
        "fw2t": fw[:, DK:].T.astype(bf),
        "pwt": pw.T.astype(bf),
        "eb": eb.reshape(DK, 1).astype(np.float32),
        "ab": ab.reshape(DK, 1).astype(np.float32),
        "fb": fb.reshape(DK, 1).astype(np.float32),
        "pb": pb.reshape(1, 1).astype(np.float32),
        "mv0r": np.repeat(mv0.T, BL, axis=1).astype(bf),
    }
    in_maps = []
    for core in range(NCORES):
        q_flat = qs[core * BL:(core + 1) * BL].T.reshape(-1)   # t-major
        c_flat = cs[core * BL:(core + 1) * BL].T.reshape(-1)
        m = dict(rep)
        m["qseq_w"] = np.ascontiguousarray(
            q_flat.reshape(BS // 16, 16).T).astype(np.int16)
        m["corrf"] = np.broadcast_to(c_flat.astype(np.float32),
                                     (4, BS)).copy()
        in_maps.append(m)
    return in_maps


def kernel(**inputs):
    global _PROG
    if _PROG is None:
        _PROG = _build_program()
    in_maps = _host_inputs(inputs)
    res = run_bass_kernel_spmd(_PROG, in_maps, core_ids=list(range(NCORES)))
    out = np.zeros((B, S), np.float32)
    for core in range(NCORES):
        o = res.results[core]["out"].reshape(S, BL)
        out[core * BL:(core + 1) * BL] = o.T
    return out


# revision 25
# speedup vs baseline: 1.9065x; 1.0430x over previous
# DKVMN Trainium2 Bass kernel (v3).
#
# Sharding: data-parallel over batch across 8 NeuronCores (8 sequences each);
# embedding tables and all parameters replicated.
#
# Per-core program (bs = t*8 + b, "t-major", BS=1600):
#   P1  q2c_table/q2c_mask rows gathered by question id (gpsimd ap_gather;
#       the gather microcode library is pre-warmed by a dummy gather so the
#       ~45us Q7 library load overlaps the input DMAs).
#   P2  index math on DVE; masked entries redirect to out-of-range ids
#       (500/1000) that no count chunk matches.
#   P3  indices/correctness/1-den broadcast to all 128 partitions via PE
#       rank-1 matmuls (ones ⊗ row) + ACT copies — no gpsimd library swaps.
#   P4  one-hot COUNT matrices by iota-compare on DVE (fp16, 4x mode) over
#       4 chunks of 125 concept rows; value-table counts derived from the
#       key counts by correctness masking (500 = 4*125 keeps chunks aligned).
#       Embedding gathers become PE matmuls over the natural-layout tables.
#   P6  w = softmax(kbar^T Mk^T), batched: one PE pass into PSUM, one exp,
#       tree-sum over slots, one reciprocal, one scaled multiply.
#   P7  e/a = sigmoid/tanh(vbar^T W^T + b) (PE + ACT), t-major contiguous.
#   P8  recurrence Mv_t = Mv_{t-1} * (1 - w e^T) + w a^T over 10 chunks of
#       20 steps, everything t-outer so every DVE op runs in 2x/4x mode:
#       PE broadcasts w rows into PSUM, ACT copies them contiguously to
#       SBUF, DVE builds A = 1 - w*e (TT+TS) batched, gpsimd builds
#       B = w*a (idle engine), then a 2-TT-per-step chain advances the
#       state in-place in a states buffer; reads are one batched multiply
#       plus an add-tree over slots, all in 2x mode.
#   P9  f = tanh([reads, kbar] f_W^T + f_b); out = sigmoid(f p_W^T + p_b).
import sys

for _p in ("/opt/trn_rl_repo", "/root/.axon_site/_ro/trn_rl_repo"):
    if _p not in sys.path:
        sys.path.append(_p)

from contextlib import ExitStack

import numpy as np
import ml_dtypes

import concourse.bass as bass
import concourse.bacc as bacc
import concourse.mybir as mybir
from concourse.bass_utils import run_bass_kernel_spmd
from concourse.tile import TileContext

F32 = mybir.dt.float32
BF16 = mybir.dt.bfloat16
FP16 = mybir.dt.float16
I32 = mybir.dt.int32
I16 = mybir.dt.int16
AF = mybir.ActivationFunctionType
OP = mybir.AluOpType

B, S, DK, SLOTS = 64, 200, 128, 50
NUM_Q, NUM_C, MAXC = 10000, 500, 4
NCORES = 8
BL = B // NCORES          # 8 sequences per core
BS = BL * S               # 1600 (bs = t*BL + b)
NB = SLOTS * BL           # 400 state columns per step (n-major, b-inner)
CP = 125                  # concept rows per table chunk (500 = 4*125)
KCH = 4                   # key table chunks
VCH = 8                   # value table chunks (1000 = 8*125)
NCH = (BS + 127) // 128   # 13 bs-chunks for softmax
TCH = 20                  # recurrence chunk length (steps)
NCHK = S // TCH           # 10 chunks

_PROG = None  # cached compiled program


def _build_program():
    nc = bacc.Bacc("TRN2", target_bir_lowering=False, debug=False,
                   num_devices=NCORES)

    def din(name, shape, dt):
        return nc.dram_tensor(name, shape, dt, kind="ExternalInput")

    qseq_w = din("qseq_w", [16, BS // 16], I16)
    corrf = din("corrf", [4, BS], F32)
    q2c_comb = din("q2c_comb", [16, 2 * NUM_Q], I16)
    ket_d = din("ket", [CP, KCH * DK], FP16)
    vet_d = din("vet", [CP, VCH * DK], FP16)
    iof_d = din("iof", [128, 1], F32)
    mkt_d = din("mkt", [DK, SLOTS], BF16)
    ewt_d = din("ewt", [DK, DK], BF16)
    awt_d = din("awt", [DK, DK], BF16)
    fw1t_d = din("fw1t", [DK, DK], BF16)
    fw2t_d = din("fw2t", [DK, DK], BF16)
    pwt_d = din("pwt", [DK, 1], BF16)
    eb_d = din("eb", [DK, 1], F32)
    ab_d = din("ab", [DK, 1], F32)
    fb_d = din("fb", [DK, 1], F32)
    pb_d = din("pb", [1, 1], F32)
    mv0_d = din("mv0r", [DK, NB], BF16)
    out_d = nc.dram_tensor("out", [1, BS], F32, kind="ExternalOutput")

    with ExitStack() as ctx:
        ctx.enter_context(
            nc.allow_low_precision("bf16 state; rel-err budget 2e-2"))
        tc = ctx.enter_context(TileContext(nc))
        const = ctx.enter_context(tc.tile_pool(name="const", bufs=1))
        main = ctx.enter_context(tc.tile_pool(name="main", bufs=1))
        dram = ctx.enter_context(tc.tile_pool(name="dram", bufs=1,
                                              space="DRAM"))

        # ---- persistent tiles ----
        kbar = main.tile([DK, BS], BF16, tag="kbar")
        e_all = main.tile([DK, BS], BF16, tag="e_all")
        a_all = main.tile([DK, BS], BF16, tag="a_all")
        idb = main.tile([DK, BS], BF16, tag="idb")
        w_rows = main.tile([128, NCH, SLOTS], BF16, tag="w_rows")
        reads_bs = main.tile([DK, BS], BF16, tag="reads_bs")
        f_all = main.tile([DK, BS], BF16, tag="f_all")
        out_sb = main.tile([1, BS], F32, tag="out_sb")

        # ---- params (const pool) ----
        kes = const.tile([CP, KCH, DK], FP16, tag="kes")
        ves = const.tile([CP, VCH, DK], FP16, tag="ves")
        iof = const.tile([128, 1], F32, tag="iof")
        mkt = const.tile([DK, SLOTS], BF16, tag="mkt")
        ewt = const.tile([DK, DK], BF16, tag="ewt")
        awt = const.tile([DK, DK], BF16, tag="awt")
        fw1t = const.tile([DK, DK], BF16, tag="fw1t")
        fw2t = const.tile([DK, DK], BF16, tag="fw2t")
        pwt = const.tile([DK, 1], BF16, tag="pwt")
        eb = const.tile([DK, 1], F32, tag="eb")
        ab = const.tile([DK, 1], F32, tag="ab")
        fb = const.tile([DK, 1], F32, tag="fb")
        pb = const.tile([1, 1], F32, tag="pb")
        ones4 = const.tile([4, 1], F32, tag="ones4")
        quarter = const.tile([4, DK], F32, tag="quarter")
        onesel = const.tile([4, 4, DK], FP16, tag="onesel")
        ones128 = const.tile([128, DK], BF16, tag="ones128")
        nc.vector.memset(ones4[...], 1.0)
        nc.vector.memset(quarter[...], 0.25)
        for j in range(4):
            nc.vector.tensor_scalar(onesel[:, j, :],
                                    iof[0:4, :].broadcast_to([4, DK]),
                                    float(j), None, op0=OP.is_equal)
        nc.vector.memset(ones128[...], 1.0)
        nc.sync.dma_start(kes[...],
                          ket_d[...].rearrange("p (c d) -> p c d", c=KCH))
        nc.sync.dma_start(ves[...],
                          vet_d[...].rearrange("p (c d) -> p c d", c=VCH))
        for tile_, dt_ in ((iof, iof_d), (mkt, mkt_d), (ewt, ewt_d),
                           (awt, awt_d), (fw1t, fw1t_d), (fw2t, fw2t_d),
                           (pwt, pwt_d), (eb, eb_d), (ab, ab_d), (fb, fb_d),
                           (pb, pb_d)):
            nc.sync.dma_start(tile_[...], dt_[...])

        # gpsimd gather-library warm-up: a dummy 16-index gather forces the
        # Q7 microcode load to overlap the big input DMAs.
        dg_t = const.tile([16, 2, 2], I16, tag="dg_t")
        dg_i = const.tile([16, 1], I16, tag="dg_i")
        dg_o = const.tile([16, 1, 2], I16, tag="dg_o")
        nc.vector.memset(dg_t[...], 0)
        nc.vector.memset(dg_i[...], 0)
        nc.gpsimd.ap_gather(dg_o[...], dg_t[...], dg_i[...], channels=16,
                            num_elems=2, d=2, num_idxs=16)

        psA_stack = ExitStack()
        psA = psA_stack.enter_context(
            tc.tile_pool(name="psA", bufs=1, space="PSUM"))

        pfB_stack = ExitStack()
        pfB = pfB_stack.enter_context(tc.tile_pool(name="pfB", bufs=1))
        kbi = pfB.tile([128, KCH, BS], FP16, tag="kbi")
        corrh = pfB.tile([128, BS], FP16, tag="corrh")
        cnt = pfB.tile([CP, KCH, BS], FP16, tag="cnt")
        wvm = pfB.tile([CP, VCH, BS], FP16, tag="wvm")
        isq = pfB.tile([CP, 4, BS], FP16, tag="isq")
        s01 = pfB.tile([CP, 2, BS], FP16, tag="s01")
        iotc = pfB.tile([CP, 1], F32, tag="iotc")
        vbar = pfB.tile([DK, BS], BF16, tag="vbar")

        with tc.tile_pool(name="pfA", bufs=1) as pfA:
            # ---- P1: gather cids/mask rows by question id ----
            q2c_t = pfA.tile([16, NUM_Q, 2], I16, tag="q2c")
            qw = pfA.tile([16, BS // 16], I16, tag="qw")
            nc.sync.dma_start(q2c_t[...], q2c_comb[...])
            nc.sync.dma_start(qw[...], qseq_w[...])
            qc = pfA.tile([16, BS, 2], I16, tag="qc")
            nc.gpsimd.ap_gather(qc[...], q2c_t[...], qw[...], channels=16,
                                num_elems=NUM_Q, d=2, num_idxs=BS)
            # warm the gpsimd vector-op library for the P8 B-builds
            dg_a = pfA.tile([1, 4], BF16, tag="dg_a")
            dg_b = pfA.tile([1, 4], BF16, tag="dg_b")
            nc.vector.memset(dg_a[...], 1.0)
            nc.gpsimd.tensor_mul(dg_b[...], dg_a[...], dg_a[...])

            # ---- P2: index math (f32, exact) ----
            corr = pfA.tile([4, BS], F32, tag="corr")
            nc.sync.dma_start(corr[...], corrf[...])
            cidsf = pfA.tile([4, BS], F32, tag="cidsf")
            mskf = pfA.tile([4, BS], F32, tag="mskf")
            nc.vector.tensor_copy(cidsf[...], qc[0:4, :, 0])
            nc.vector.tensor_copy(mskf[...], qc[0:4, :, 1])
            # masked entries -> id 500: outside every 125-row chunk, so
            # they contribute no counts
            k1 = pfA.tile([4, BS], F32, tag="k1")
            nc.vector.scalar_tensor_tensor(k1[...], cidsf[...], -500.0,
                                           mskf[...], op0=OP.add, op1=OP.mult)
            kh = pfA.tile([4, BS], FP16, tag="kh")
            nc.vector.tensor_scalar_add(kh[...], k1[...], 500.0)

            # den = max(sum_j mask, 1); inv_bf = 1/den
            inv_bf = pfA.tile([1, BS], BF16, tag="inv_bf")
            for c in range(4):
                sl = slice(c * 400, (c + 1) * 400)
                msum_ps = psA.tile([1, 400], F32, tag="mm1", bufs=2,
                                   name=f"msum{c}")
                nc.tensor.matmul(msum_ps[...], ones4[...], mskf[:, sl])
                den_c = pfA.tile([1, 400], F32, tag="den", bufs=2,
                                 name=f"den{c}")
                nc.vector.tensor_scalar_max(den_c[...], msum_ps[...], 1.0)
                nc.vector.reciprocal(inv_bf[:, sl], den_c[...])

            # ---- P3: broadcasts via PE rank-1 matmuls + ACT copies ----
            for s in range(4):
                sl = slice(s * 400, (s + 1) * 400)
                for j in range(4):
                    kp = psA.tile([128, 400], F32, tag="mm2", bufs=4)
                    nc.tensor.matmul(kp[...], onesel[:, j, :],
                                     kh[:, sl])
                    nc.scalar.activation(kbi[:, j, sl], kp[...], AF.Copy)
                cp_ = psA.tile([128, 400], F32, tag="mm2", bufs=4)
                nc.tensor.matmul(cp_[...], quarter[...], corr[:, sl])
                nc.scalar.activation(corrh[:, sl], cp_[...], AF.Copy)
                ip_ = psA.tile([128, 400], F32, tag="mm2", bufs=4)
                nc.tensor.matmul(ip_[...], ones128[0:1, :], inv_bf[:, sl])
                nc.scalar.activation(idb[:, sl], ip_[...], AF.Copy)

        # ---- P4: count matrices by iota-compare; PE "gathers" ----
        kbi3 = kbi[0:CP, :, :]
        for c in range(KCH):
            nc.vector.tensor_scalar_add(iotc[...], iof[0:CP, :],
                                        float(CP * c))
            nc.vector.tensor_scalar(isq[...], kbi3, iotc[...], None,
                                    op0=OP.is_equal)
            nc.vector.tensor_add(s01[...], isq[:, 0:2, :], isq[:, 2:4, :])
            nc.vector.tensor_add(cnt[:, c, :], s01[:, 0, :], s01[:, 1, :])
        for c in range(KCH):
            # value counts: chunk c gets correct=0 mass, chunk 4+c correct=1
            nc.vector.tensor_mul(wvm[:, KCH + c, :], cnt[:, c, :],
                                 corrh[0:CP, :])
            nc.vector.tensor_sub(wvm[:, c, :], cnt[:, c, :],
                                 wvm[:, KCH + c, :])

        for s in range(4):
            sl = slice(s * 400, (s + 1) * 400)
            kb_ps = psA.tile([DK, 400], F32, tag="mm2", bufs=4)
            for c in range(KCH):
                nc.tensor.matmul(kb_ps[...], kes[:, c, :], cnt[:, c, sl],
                                 start=(c == 0), stop=(c == KCH - 1))
            nc.vector.tensor_mul(kbar[:, sl], kb_ps[...], idb[:, sl])
            vb_ps = psA.tile([DK, 400], F32, tag="mm2", bufs=4)
            for c in range(VCH):
                nc.tensor.matmul(vb_ps[...], ves[:, c, :], wvm[:, c, sl],
                                 start=(c == 0), stop=(c == VCH - 1))
            nc.vector.tensor_mul(vbar[:, sl], vb_ps[...], idb[:, sl])

        # ---- P6: w = softmax(kbar^T @ Mk^T), batched ----
        # 64-slot padding keeps every matmul output inside one PSUM bank
        lg = psA.tile([128, NCH, 64], F32, tag="mm3", bufs=1)
        for c in range(NCH):
            p = min(128, BS - c * 128)
            nc.tensor.matmul(lg[:p, c, 0:SLOTS],
                             kbar[:, c * 128:c * 128 + p], mkt[...])
        ex = pfB.tile([128, NCH, SLOTS], F32, tag="ex")
        nc.scalar.activation(ex[...], lg[:, :, 0:SLOTS], AF.Exp)
        t25 = pfB.tile([128, NCH, 25], F32, tag="t25")
        t12 = pfB.tile([128, NCH, 12], F32, tag="t12")
        t6 = pfB.tile([128, NCH, 6], F32, tag="t6")
        t3 = pfB.tile([128, NCH, 3], F32, tag="t3")
        sx = pfB.tile([128, NCH, 1], F32, tag="sx")
        rx = pfB.tile([128, NCH], F32, tag="rx")
        nc.vector.tensor_add(t25[...], ex[:, :, 0:25], ex[:, :, 25:50])
        nc.vector.tensor_add(t12[...], t25[:, :, 0:12], t25[:, :, 12:24])
        nc.vector.tensor_add(t6[...], t12[:, :, 0:6], t12[:, :, 6:12])
        nc.vector.tensor_add(t3[...], t6[:, :, 0:3], t6[:, :, 3:6])
        nc.vector.tensor_add(sx[...], t3[:, :, 0:1], t3[:, :, 1:2])
        nc.vector.tensor_add(sx[...], sx[...], t3[:, :, 2:3])
        nc.vector.tensor_add(sx[...], sx[...], t25[:, :, 24:25])
        nc.vector.reciprocal(rx[...], sx[:, :, 0])
        nc.vector.tensor_tensor(
            w_rows[...], ex[...],
            rx[...].unsqueeze(2).broadcast_to([128, NCH, SLOTS]), OP.mult)

        # reorder w into per-step rows via DRAM bounce (loaded per chunk)
        wdram = dram.tile([NCH * 128 * SLOTS], BF16, tag="wdram")
        nc.sync.dma_start(
            wdram[...].rearrange("(c p n) -> p c n", p=128, n=SLOTS),
            w_rows[...])

        # ---- P7: e/a (t-major contiguous) ----
        for c in range(4):
            sl = slice(c * 400, (c + 1) * 400)
            ep = psA.tile([DK, 400], F32, tag="mm2", bufs=4)
            nc.tensor.matmul(ep[...], ewt[...], vbar[:, sl])
            nc.scalar.activation(e_all[:, sl], ep[...], AF.Sigmoid,
                                 bias=eb[...], scale=1.0)
            ap_ = psA.tile([DK, 400], F32, tag="mm2", bufs=4)
            nc.tensor.matmul(ap_[...], awt[...], vbar[:, sl])
            nc.scalar.activation(a_all[:, sl], ap_[...], AF.Tanh,
                                 bias=ab[...], scale=1.0)

        pfB_stack.close()
        psA_stack.close()

        # ---- P8: recurrence, t-outer chunked chain ----
        with ExitStack() as rstk:
            pr = rstk.enter_context(tc.tile_pool(name="pr", bufs=1))
            psW = rstk.enter_context(
                tc.tile_pool(name="psW", bufs=1, space="PSUM"))

            w32c = [pr.tile([128, 8, NB], BF16, tag=f"w32c{i}",
                            name=f"w32c{i}") for i in range(2)]
            wsb = [pr.tile([128, TCH * NB], BF16, tag=f"wsb{i}",
                           name=f"wsb{i}") for i in range(2)]
            A2 = [pr.tile([128, TCH * NB], BF16, tag=f"A2{i}",
                          name=f"A2{i}") for i in range(2)]
            B2 = [pr.tile([128, TCH * NB], BF16, tag=f"B2{i}",
                          name=f"B2{i}") for i in range(2)]
            st = pr.tile([128, (TCH + 1) * NB], BF16, tag="st")
            scr = pr.tile([128, TCH * NB], BF16, tag="scr")
            p0t = pr.tile([128, TCH * NB], BF16, tag="p0t")
            m2 = [pr.tile([128, NB], BF16, tag=f"m2{i}", name=f"m2{i}")
                  for i in range(2)]
            mv0s = pr.tile([DK, NB], BF16, tag="mv0s")
            nc.sync.dma_start(mv0s[...], mv0_d[...])
            nc.vector.tensor_copy(st[:, 0:NB], mv0s[...])

            e3 = e_all[...].rearrange("p (t b) -> p t b", b=BL)
            a3 = a_all[...].rearrange("p (t b) -> p t b", b=BL)

            for k in range(NCHK):
                wk = wsb[k % 2]
                Ak, Bk, wc = A2[k % 2], B2[k % 2], w32c[k % 2]
                u0 = (k * TCH) // 3
                for k3 in range(3):
                    base = (3 * u0 + k3) * NB
                    span = min(8 * 3 * NB, NCH * 128 * SLOTS - base)
                    nu = span // (3 * NB)
                    src = wdram[base:base + nu * 3 * NB] \
                        .rearrange("(u j bn) -> u j bn", j=3, bn=NB)[:, 0, :]
                    nc.sync.dma_start(wc[32 * k3:32 * k3 + 1, 0:nu, :], src)
                # PE: broadcast w rows for 20 steps (5 PSUM groups of 4)
                for g in range(TCH // 4):
                    wbps = psW.tile([128, 4 * 512], F32, tag="wbps", bufs=2,
                                    name=f"wbps{k}_{g}")
                    for s4 in range(4):
                        t = k * TCH + g * 4 + s4
                        al = 32 * (t % 3)
                        nc.tensor.matmul(
                            wbps[:, 512 * s4:512 * s4 + NB],
                            ones128[al:al + 1, :],
                            wc[al:al + 1, t // 3 - u0, :])
                    nc.scalar.activation(
                        wk[:, g * 4 * NB:(g + 1) * 4 * NB]
                        .rearrange("p (s x) -> p s x", s=4),
                        wbps[...].rearrange("p (s x) -> p s x",
                                            x=512)[:, :, 0:NB],
                        AF.Copy)

                tv = slice(k * TCH, (k + 1) * TCH)
                ebc = e3[:, tv, :].unsqueeze(2).broadcast_to(
                    [128, TCH, SLOTS, BL])
                abc = a3[:, tv, :].unsqueeze(2).broadcast_to(
                    [128, TCH, SLOTS, BL])
                wk3 = wk[...].rearrange("p (t n b) -> p t n b", n=SLOTS, b=BL)
                scr3 = scr[...].rearrange("p (t n b) -> p t n b",
                                          n=SLOTS, b=BL)
                Bk3 = Bk[...].rearrange("p (t n b) -> p t n b", n=SLOTS, b=BL)
                # A = 1 - w*e (DVE), B = w*a (gpsimd)
                nc.vector.tensor_tensor(scr3, wk3, ebc, OP.mult)
                nc.vector.tensor_scalar(Ak[...], scr[...], -1.0, 1.0,
                                        op0=OP.mult, op1=OP.add)
                nc.gpsimd.tensor_mul(Bk3, wk3, abc)
                if k > 0:
                    nc.vector.tensor_copy(st[:, 0:NB],
                                          st[:, TCH * NB:(TCH + 1) * NB])
                for j in range(TCH):
                    mj = m2[j % 2]
                    nc.vector.tensor_tensor(mj[...], st[:, j * NB:(j + 1) * NB],
                                            Ak[:, j * NB:(j + 1) * NB],
                                            OP.mult)
                    nc.vector.tensor_tensor(st[:, (j + 1) * NB:(j + 2) * NB],
                                            mj[...],
                                            Bk[:, j * NB:(j + 1) * NB],
                                            OP.add)
                # reads: p0 = w_t * Mv_{t-1}, then add-tree over n
                nc.vector.tensor_tensor(p0t[...], st[:, 0:TCH * NB],
                                        wk[...], OP.mult)
                p03 = p0t[...].rearrange("p (t n b) -> p t n b",
                                         n=SLOTS, b=BL)
                q1v = scr[:, 0:4000].rearrange("p (t n b) -> p t n b",
                                               n=25, b=BL)
                q2v = scr[:, 4000:5920].rearrange("p (t n b) -> p t n b",
                                                  n=12, b=BL)
                q3v = scr[:, 5920:6880].rearrange("p (t n b) -> p t n b",
                                                  n=6, b=BL)
                q4v = scr[:, 6880:7360].rearrange("p (t n b) -> p t n b",
                                                  n=3, b=BL)
                q5v = scr[:, 7360:7520].rearrange("p (t b) -> p t b", b=BL)
                q6v = scr[:, 7520:7680].rearrange("p (t b) -> p t b", b=BL)
                nc.vector.tensor_add(q1v, p03[:, :, 0:25, :],
                                     p03[:, :, 25:50, :])
                nc.vector.tensor_add(q2v, q1v[:, :, 0:12, :],
                                     q1v[:, :, 12:24, :])
                nc.vector.tensor_add(q3v, q2v[:, :, 0:6, :],
                                     q2v[:, :, 6:12, :])
                nc.vector.tensor_add(q4v, q3v[:, :, 0:3, :],
                                     q3v[:, :, 3:6, :])
                nc.vector.tensor_add(q5v, q4v[:, :, 0, :], q4v[:, :, 1, :])
                nc.vector.tensor_add(q6v, q5v, q4v[:, :, 2, :])
                nc.vector.tensor_add(
                    reads_bs[:, k * TCH * BL:(k + 1) * TCH * BL]
                    .rearrange("p (t b) -> p t b", b=BL),
                    q6v, q1v[:, :, 24, :])

        # ---- P9: output head ----
        psB_stack = ExitStack()
        psB = psB_stack.enter_context(
            tc.tile_pool(name="psB", bufs=1, space="PSUM"))
        for c in range(4):
            sl = slice(c * 400, (c + 1) * 400)
            fp = psB.tile([DK, 400], F32, tag="mm2", bufs=4)
            nc.tensor.matmul(fp[...], fw1t[...], reads_bs[:, sl],
                             start=True, stop=False)
            nc.tensor.matmul(fp[...], fw2t[...], kbar[:, sl],
                             start=False, stop=True)
            nc.scalar.activation(f_all[:, sl], fp[...], AF.Tanh,
                                 bias=fb[...], scale=1.0)
        for c in range(4):
            sl = slice(c * 400, (c + 1) * 400)
            pp = psB.tile([1, 400], F32, tag="mm1", bufs=2)
            nc.tensor.matmul(pp[...], pwt[...], f_all[:, sl])
            nc.scalar.activation(out_sb[:, sl], pp[...], AF.Sigmoid,
                                 bias=pb[...], scale=1.0)
        nc.sync.dma_start(out_d[...], out_sb[...])
        psB_stack.close()

    nc.finalize()
    return nc


def _host_inputs(inputs):
    """Build per-core + replicated DRAM inputs from the full problem inputs."""
    bf = ml_dtypes.bfloat16
    qs = np.asarray(inputs["question_seq"]).astype(np.int64)
    cs = np.asarray(inputs["correctness_seq"]).astype(np.int64)
    q2c = np.asarray(inputs["q2c_table"]).astype(np.int32)
    q2m = np.asarray(inputs["q2c_mask"]).astype(np.int32)
    ke = np.asarray(inputs["key_embed"], np.float32)
    ve = np.asarray(inputs["value_embed"], np.float32)
    mk = np.asarray(inputs["Mk"], np.float32)
    mv0 = np.asarray(inputs["Mv0"], np.float32)
    fw = np.asarray(inputs["f_W"], np.float32)
    fb = np.asarray(inputs["f_b"], np.float32)
    ew = np.asarray(inputs["e_W"], np.float32)
    eb = np.asarray(inputs["e_b"], np.float32)
    aw = np.asarray(inputs["a_W"], np.float32)
    ab = np.asarray(inputs["a_b"], np.float32)
    pw = np.asarray(inputs["p_W"], np.float32)
    pb = np.asarray(inputs["p_b"], np.float32)

    # [CP, C*DK] chunked-contiguous table layouts (chunk c rows 125c..125c+124)
    kep = ke.astype(np.float16).reshape(KCH, CP, DK).transpose(1, 0, 2) \
        .reshape(CP, KCH * DK)
    vep = ve.astype(np.float16).reshape(VCH, CP, DK).transpose(1, 0, 2) \
        .reshape(CP, VCH * DK)

    rep = {
        "q2c_comb": np.concatenate(
            [np.stack([q2c.T, q2m.T], 2).reshape(4, 2 * NUM_Q),
             np.zeros((12, 2 * NUM_Q), np.int64)], 0
        ).astype(np.int16),
        "ket": np.ascontiguousarray(kep),
        "vet": np.ascontiguousarray(vep),
        "iof": np.arange(128, dtype=np.float32).reshape(128, 1),
        "mkt": mk.T.astype(bf),
        "ewt": ew.T.astype(bf),
        "awt": aw.T.astype(bf),
        "fw1t": fw[:, :DK].T.astype(bf),
        "fw2t": fw[:, DK:].T.astype(bf),
        "pwt": pw.T.astype(bf),
        "eb": eb.reshape(DK, 1).astype(np.float32),
        "ab": ab.reshape(DK, 1).astype(np.float32),
        "fb": fb.reshape(DK, 1).astype(np.float32),
        "pb": pb.reshape(1, 1).astype(np.float32),
        "mv0r": np.repeat(mv0.T, BL, axis=1).astype(bf),
    }
    in_maps = []
    for core in range(NCORES):
        q_flat = qs[core * BL:(core + 1) * BL].T.reshape(-1)   # t-major
        c_flat = cs[core * BL:(core + 1) * BL].T.reshape(-1)
        m = dict(rep)
        m["qseq_w"] = np.ascontiguousarray(
            q_flat.reshape(BS // 16, 16).T).astype(np.int16)
        m["corrf"] = np.broadcast_to(c_flat.astype(np.float32),
                                     (4, BS)).copy()
        in_maps.append(m)
    return in_maps


def kernel(**inputs):
    global _PROG
    if _PROG is None:
        _PROG = _build_program()
    in_maps = _host_inputs(inputs)
    res = run_bass_kernel_spmd(_PROG, in_maps, core_ids=list(range(NCORES)))
    out = np.zeros((B, S), np.float32)
    for core in range(NCORES):
        o = res.results[core]["out"].reshape(S, BL)
        out[core * BL:(core + 1) * BL] = o.T
    return out
